# revision 17
# baseline (speedup 1.0000x reference)
"""BitLinear (ternary 2-bit weights, group-128 scales, dynamic int8 activation
quant) for Trainium2, tensor-parallel over 8 NeuronCores (shard N).

Math (per core, N-shard NS):
  s[m]   = 127 / clip(max_k |x[m,k]|, 1e-5)
  q[m,k] = round(x[m,k] * s[m])                      (integers in [-127,127])
  w[n,k] in {-1,0,1} from 2-bit codes c=w+1, 4 codes/byte
  out[m,n] = (sum_k q[m,k] * w[n,k] * ws[n, k//128]) / s[m]   -> bf16

Pipelined schedule (v2): x chunks DMA first (tail chunks split for finer
arrival granularity); row abs-max runs chunk-pipelined on DVE interleaved
with the weight decode (tensor_scalar extract at 4x + tensor_tensor
scale-multiply at 2x; several multiplies offloaded to the otherwise-idle
Pool engine); quantization (ACT fma-magic RNE rounding) + PE
identity-transposes + main matmuls are pipelined per 2048-wide k-chunk so
the GEMM stream starts ~26us instead of ~50us.  The m-block mh=1 lags mh=0
and catches up inside the main stream.  Transposes run in is_transpose mode
into bf16 PSUM tiles, evacuated in [128,1024] batches into a single qp_all
buffer.  The "-1" correction (out -= gz @ ws.T, gz = per-group sums of q)
accumulates 8-column gmat matmuls in PE decode-stall slots, is transposed on
the PE, and lands as 4 bf16 matmuls into the persistent PSUM accumulators
mid-stream.  fp32 warm-up matmuls tied to x-chunk arrivals keep the PE HAM
activity monitor at 2.4GHz through the head.
"""

import sys

import numpy as np

try:
    import concourse.bass as bass
except ImportError:  # fresh grading dir: fall back to the repo checkout
    sys.path.insert(0, "/opt/trn_rl_repo")
    import concourse.bass as bass

import ml_dtypes

import concourse.mybir as mybir
import concourse.tile as tile
from concourse import bacc, bass_utils
from concourse.masks import make_identity

FP32 = mybir.dt.float32
BF16 = mybir.dt.bfloat16
U16 = mybir.dt.uint16
MAGIC = float(2 << 22)  # 2^23
COPY = mybir.ActivationFunctionType.Copy

M, N, K, GS = 256, 8192, 8192, 128
NCORES = 8


def build_nc(m=M, k=K, ns=N // NCORES):
    """One core's program: full m,k; n-shard of size ns."""
    kh = k // 8          # uint16 count along K
    kb = kh // 128       # kh-blocks of 128 partitions
    st_n = kb // 2       # supertiles = pairs of kh-blocks
    mt = m // 128        # m partition-tiles
    ck = k // 2048       # 2048-wide k-chunks (256 kh, 2 kh-blocks)
    nsl = min(512, ns)   # matmul rhs free-dim slice
    nh_n = ns // nsl
    g_n = k // GS        # scale groups along K
    nslots = st_n * 8    # linear (st, t) slot index space
    assert mt == 2 and ck == st_n == 4

    nc = bacc.Bacc()
    x_d = nc.declare_dram_parameter("x", [m, k], FP32, isOutput=False)
    w_d = nc.declare_dram_parameter("w16", [kh, ns], U16, isOutput=False)
    se_d = nc.declare_dram_parameter("sexp", [kh, ns], BF16, isOutput=False)
    # -ws.T in bf16 for the "-1" correction matmul (same bf16 rounding as
    # sexp so the w=0 contributions cancel exactly)
    sn_d = nc.declare_dram_parameter("sneg", [g_n, ns], BF16, isOutput=False)
    # G[kh, 8*t+gl] = (kh//16 == gl): group-sum indicator, bf16
    gm_d = nc.declare_dram_parameter("gmat", [128, 64], BF16, isOutput=False)
    out_d = nc.declare_dram_parameter("out", [m, ns], BF16, isOutput=True)

    x_r = x_d.rearrange("(T p) k -> T p k", p=128)          # [mt,128,k]
    w_r = w_d.rearrange("(B p) n -> p B n", p=128)          # [128,kb,ns]
    se_r = se_d.rearrange("(B p) n -> p B n", p=128)        # [128,kb,ns]
    out_r = out_d.rearrange("(T p) n -> T p n", p=128)      # [mt,128,ns]

    # Pool shares SBUF ports with DVE: concurrent Pool bulk ops slow DVE
    # ~6x (measured), so all decode work stays on DVE.

    with tile.TileContext(nc) as tc:
        with (
            tc.tile_pool(name="const", bufs=1) as constp,
            tc.tile_pool(name="stat", bufs=1) as statp,
            tc.tile_pool(name="xp", bufs=7) as xp,
            tc.tile_pool(name="t1p", bufs=2) as t1p,
            tc.tile_pool(name="qcp", bufs=2) as qcp,
            tc.tile_pool(name="wse", bufs=2) as wsep,
            tc.tile_pool(name="cpp", bufs=2) as cpp,
            tc.tile_pool(name="wsp", bufs=10) as wsp,
            tc.tile_pool(name="ob", bufs=2) as obp,
            tc.tile_pool(name="psm", bufs=1, space="PSUM") as psmp,
            tc.tile_pool(name="psT", bufs=2, space="PSUM") as psTp,
            tc.tile_pool(name="pss", bufs=1, space="PSUM") as pssp,
        ):
            # -------- DMA issues: x(0,*) first, st0 weights, then x(1,*) --
            xsb = {}

            def x_dma(mh, c, pieces=1):
                t_ = xp.tile([128, 2048], FP32, tag="x", name=f"x{mh}{c}")
                xsb[(mh, c)] = t_
                step = 2048 // pieces
                for p_ in range(pieces):
                    sl = slice(step * p_, step * (p_ + 1))
                    nc.sync.dma_start(
                        t_[:, sl], x_r[mh, :, 2048 * c + step * p_ :
                                       2048 * c + step * (p_ + 1)])

            wt_t, se_t = {}, {}

            def load_st(sti, eng=None):
                eng = eng or nc.sync
                wt = wsep.tile([128, 2 * ns], U16, tag="w16", name=f"wt{sti}")
                se = wsep.tile([128, 2 * ns], BF16, tag="sexp",
                               name=f"se{sti}")
                wt3 = wt.rearrange("p (B n) -> p B n", B=2)
                se3 = se.rearrange("p (B n) -> p B n", B=2)
                eng.dma_start(wt3[:], w_r[:, 2 * sti : 2 * sti + 2, :])
                eng.dma_start(se3[:], se_r[:, 2 * sti : 2 * sti + 2, :])
                wt_t[sti], se_t[sti] = wt, se

            x_dma(0, 0)
            x_dma(0, 1)
            x_dma(0, 2)
            x_dma(0, 3, pieces=4)
            load_st(0)
            x_dma(1, 0)
            x_dma(1, 1)
            x_dma(1, 2)
            x_dma(1, 3, pieces=2)
            sneg = constp.tile([g_n, ns], BF16, tag="sneg")
            nc.sync.dma_start(sneg[:], sn_d[:])
            gmat = constp.tile([128, 64], BF16, tag="gmat")
            nc.sync.dma_start(gmat[:], gm_d[:])
            load_st(1)
            load_st(2)
            load_st(3)

            # ---------------- constants ----------------------------------
            ident = constp.tile([128, 128], BF16, tag="ident")
            make_identity(nc, ident)

            # ------------- stats: row abs-max + recip, all on DVE ---------
            rpart = [statp.tile([128, 16], FP32, tag=f"rp{t}", name=f"rp{t}")
                     for t in range(mt)]
            rmax = [statp.tile([128, 1], FP32, tag=f"rm{t}", name=f"rm{t}")
                    for t in range(mt)]
            s_pp = [statp.tile([128, 1], FP32, tag=f"sp{t}", name=f"sp{t}")
                    for t in range(mt)]
            r1s = [statp.tile([128, 1], FP32, tag=f"rs{t}", name=f"rs{t}")
                   for t in range(mt)]
            rcols = [0, 0]

            def rm_chunk(mh, c, pieces=1):
                step = 2048 // pieces
                for h in range(pieces):
                    sl = slice(step * h, step * (h + 1))
                    nc.vector.tensor_reduce(
                        rpart[mh][:, rcols[mh] : rcols[mh] + 1],
                        xsb[(mh, c)][:, sl],
                        axis=mybir.AxisListType.X, op=mybir.AluOpType.max,
                        apply_absolute_value=True,
                    )
                    rcols[mh] += 1

            def stats_finish(mh):
                nc.vector.tensor_reduce(
                    rmax[mh][:], rpart[mh][:, : rcols[mh]],
                    axis=mybir.AxisListType.X, op=mybir.AluOpType.max,
                )
                nc.vector.tensor_scalar_max(rmax[mh][:], rmax[mh][:], 1e-5)
                nc.vector.reciprocal(s_pp[mh][:], rmax[mh][:])
                nc.vector.tensor_scalar_mul(s_pp[mh][:], s_pp[mh][:], 127.0)
                nc.vector.tensor_scalar_mul(r1s[mh][:], rmax[mh][:],
                                            1.0 / 127.0)

            # ---------------- ACT quant ----------------------------------
            qc = {}

            def quant(mh, c):
                q_ = qcp.tile([128, 2048], BF16, tag="qc", name=f"q{mh}{c}")
                qc[(mh, c)] = q_
                for h in range(2):
                    sl = slice(1024 * h, 1024 * (h + 1))
                    t1 = t1p.tile([128, 1024], FP32, tag="t1", name="t1")
                    nc.scalar.activation(t1[:], xsb[(mh, c)][:, sl], COPY,
                                         bias=MAGIC, scale=s_pp[mh][:])
                    nc.scalar.activation(q_[:, sl], t1[:], COPY, bias=-MAGIC)

            # ---------------- PE helpers ----------------------------------
            # qp_all[p, 2048*b+256*t+128*mh+mm] = q[128*mh+mm, 8*(128b+p)+t]
            qp_all = constp.tile([128, 2048 * kb], BF16, tag="qp")
            qp_r = qp_all.rearrange("p (b t m2 mm) -> p b t m2 mm",
                                    b=kb, t=8, m2=mt, mm=128)

            def warm_x(mh, c, n_mm=4):
                # scratch warms into the psgm bank (b2 resets it via start=T)
                for j in range(n_mm):
                    nc.tensor.matmul(psgm[:, :128], xsb[(mh, c)][:, :128],
                                     xsb[(mh, c)][:, :128],
                                     start=True, stop=True)

            def warm_q(mh, c, n_mm=2):
                for j in range(n_mm):
                    nc.tensor.matmul(psgm[:, :128], qc[(mh, c)][:, :128],
                                     ident[:], start=True, stop=True)

            def transpose_group(mh, c, bh2):
                """8 bit-plane transposes of one kh-block into bf16 PSUM,
                then one batched ACT evacuation into qp_all."""
                b = 2 * c + bh2
                psT = psTp.tile([128, 1024], BF16, tag="psT", name="psT")
                qv = qc[(mh, c)].rearrange("p (B kk t) -> p B t kk", B=2,
                                           kk=128, t=8)
                for t in range(8):
                    nc.tensor.transpose(psT[:, 128 * t : 128 * (t + 1)],
                                        qv[:, bh2, t, :], ident[:])
                psT3 = psT.rearrange("p (t mm) -> p t mm", t=8)
                nc.scalar.activation(qp_r[:, b, :, mh, :], psT3[:], COPY)

            def decode(sti, t):
                """c_t = (w16 >> 2t) & 3 on DVE (4x); ws_t = c_t * sexp."""
                cp = cpp.tile([128, 2 * ns], U16, tag="cp", name="cp")
                nc.vector.tensor_scalar(
                    cp[:], wt_t[sti][:], 2 * t, 3,
                    mybir.AluOpType.logical_shift_right,
                    mybir.AluOpType.bitwise_and,
                )
                ws = wsp.tile([128, 2 * ns], BF16, tag="ws", name="ws")
                nc.vector.tensor_tensor(ws[:], cp[:], se_t[sti][:],
                                        mybir.AluOpType.mult)
                ws_tiles[(sti, t)] = (ws, 0)

            # persistent PSUM accumulators
            psm = [
                [psmp.tile([128, nsl], FP32, tag=f"ps{mh}{nh}",
                           name=f"ps{mh}{nh}") for nh in range(nh_n)]
                for mh in range(mt)
            ]
            psgm = pssp.tile([128, 64 * mt], FP32, tag="psgm")
            gsqm = [constp.tile([128, 64], BF16, tag=f"gsqm{mh}",
                                name=f"gsqm{mh}") for mh in range(mt)]
            psg = pssp.tile([64, 128 * mt], BF16, tag="psg")
            gsq = constp.tile([64, 128 * mt], BF16, tag="gsq")

            ws_tiles = {}

            def main_mms(sti, t, mh, stop=False):
                ws, off = ws_tiles[(sti, t)]
                for bh in range(2):
                    b = 2 * sti + bh
                    first = sti == 0 and t == 0 and bh == 0
                    last = stop and bh == 1
                    for nh in range(nh_n):
                        nc.tensor.matmul(
                            psm[mh][nh][:],
                            qp_r[:, b, t, mh, :],
                            ws[:, off + ns * bh + nsl * nh :][:, :nsl],
                            start=first, stop=last,
                        )

            def b2_unit(mh, b):
                for t in range(8):
                    nc.tensor.matmul(
                        psgm[:, 64 * mh + 8 * b : 64 * mh + 8 * b + 8],
                        qp_r[:, b, t, mh, :], gmat[:, 8 * t : 8 * t + 8],
                        start=(t == 0), stop=(t == 7),
                    )

            def corr_chain():
                # psgm -> bf16 -> PE transpose -> gsq[g, m]; then 4 bf16
                # matmuls accumulate -gz @ ws.T into psm (start=F, stop=F)
                for mh in range(mt):
                    nc.scalar.activation(gsqm[mh][:],
                                         psgm[:, 64 * mh : 64 * mh + 64],
                                         COPY)
                    nc.tensor.transpose(psg[:, 128 * mh : 128 * (mh + 1)],
                                        gsqm[mh][:], ident[:])
                nc.scalar.activation(gsq[:], psg[:], COPY)
                for mh in range(mt):
                    for nh in range(nh_n):
                        nc.tensor.matmul(
                            psm[mh][nh][:],
                            gsq[:, 128 * mh : 128 * (mh + 1)],
                            sneg[:, nsl * nh :][:, :nsl],
                            start=False, stop=False,
                        )

            def out_evac(mh):
                for nh in range(nh_n):
                    ob = obp.tile([128, nsl], BF16, tag="ob", name="ob")
                    nc.scalar.activation(ob[:], psm[mh][nh][:], COPY,
                                         scale=r1s[mh][:])
                    nc.sync.dma_start(
                        out_r[mh, :, nsl * nh : nsl * (nh + 1)], ob[:])

            # ---------------- head emission -------------------------------
            # DVE: mh0 rowmax chunk-pipelined, stats, first decode
            rm_chunk(0, 0)
            rm_chunk(0, 1)
            rm_chunk(0, 2)
            rm_chunk(0, 3, pieces=4)
            stats_finish(0)
            # ACT: quantize chunk 0; PE: warms + first transposes
            quant(0, 0)
            for c in range(ck):
                warm_x(0, c, 3)
            warm_q(0, 0, 2)
            transpose_group(0, 0, 0)
            transpose_group(0, 0, 1)
            quant(0, 1)
            decode(0, 0)

            # ---------------- schedule tables (lin = 8*st + t) ------------
            pe_prep = {3: [(0, 1)], 6: [(0, 2)], 9: [(0, 3)],
                       12: [(1, 0)], 15: [(1, 1)], 18: [(1, 2)],
                       21: [(1, 3)]}
            act_prep = {1: [(0, 2)], 4: [(0, 3)], 8: [(1, 0)],
                        10: [(1, 1)], 12: [(1, 2)], 14: [(1, 3)]}
            dve_prep = {1: [(1, 0, 1)], 3: [(1, 1, 1)], 4: [(1, 2, 1)],
                        5: [(1, 3, 2)]}
            b2_sched = {4: [(0, 0)], 5: [(0, 1)], 7: [(0, 2)],
                        8: [(0, 3)], 10: [(0, 4)], 11: [(0, 5)],
                        13: [(0, 6), (0, 7)],
                        15: [(1, 0), (1, 1)], 18: [(1, 2), (1, 3)],
                        21: [(1, 4), (1, 5)], 23: [(1, 6), (1, 7)]}
            corr_lin = 24

            mh1_units = [(st, t) for st in range(st_n) for t in range(8)]
            mh1_done = 0
            tgroups1 = [0]

            def mh1_target(lin):
                if lin < 13:
                    return 0
                return min(nslots, (lin - 13) * 5 // 2, lin + 1,
                           8 * tgroups1[0])

            def emit_mh1(upto):
                nonlocal mh1_done
                while mh1_done < upto:
                    st_, t_ = mh1_units[mh1_done]
                    main_mms(st_, t_, 1,
                             stop=(st_ == st_n - 1 and t_ == 7))
                    mh1_done += 1

            # ---------------- main loop -----------------------------------
            for lin in range(nslots):
                sti, t = lin // 8, lin % 8
                for (mh_, c_, pieces) in dve_prep.get(lin, ()):
                    rm_chunk(mh_, c_, pieces)
                    if (mh_, c_) == (1, 3):
                        stats_finish(1)
                for (mh_, c_) in act_prep.get(lin, ()):
                    quant(mh_, c_)
                for (mh_, c_) in pe_prep.get(lin, ()):
                    transpose_group(mh_, c_, 0)
                    transpose_group(mh_, c_, 1)
                    if mh_ == 1:
                        tgroups1[0] += 1
                if (sti, t) not in ws_tiles:
                    decode(sti, t)
                main_mms(sti, t, 0, stop=(sti == st_n - 1 and t == 7))
                for (mh_, b_) in b2_sched.get(lin, ()):
                    b2_unit(mh_, b_)
                if lin == corr_lin:
                    corr_chain()
                emit_mh1(mh1_target(lin))
            out_evac(0)   # psm0 stopped at (st3,t7,mh0); overlaps trailing
            emit_mh1(nslots)
            out_evac(1)

    nc.compile()
    return nc


def host_prep(input, weight_scale, weight, ns):
    """Shard + relayout inputs for each core. Pure relayout of static weight
    data (transpose, uint8->uint16 view, group-scale expansion) plus fp32
    activation passthrough."""
    n = weight.shape[0]
    x = np.ascontiguousarray(input, dtype=np.float32)
    w_bytes = weight.astype(np.uint8)              # [N, K/4] packed bytes
    w16 = w_bytes.view(np.uint16)                  # [N, K/8] 8 codes each
    ws2 = np.asarray(weight_scale, dtype=np.float32).reshape(n, -1)  # [N,K/GS]
    ws2_b = ws2.astype(ml_dtypes.bfloat16)
    # gmat[kh, 8*t+gl] = (kh//16 == gl)  group-sum indicator
    gmat = np.zeros((128, 64), dtype=np.float32)
    for t in range(8):
        for khp in range(128):
            gmat[khp, 8 * t + khp // 16] = 1.0
    gmat = gmat.astype(ml_dtypes.bfloat16)
    in_maps = []
    for c in range(n // ns):
        sl = slice(c * ns, (c + 1) * ns)
        w16_c = np.ascontiguousarray(w16[sl].T)    # [KH, ns]
        se_c = np.ascontiguousarray(ws2_b[sl].T.repeat(16, axis=0))  # [KH,ns]
        sn_c = -np.ascontiguousarray(ws2_b[sl].T)  # [K/GS, ns] bf16
        in_maps.append(
            {"x": x, "w16": w16_c, "sexp": se_c, "sneg": sn_c, "gmat": gmat}
        )
    return in_maps


_NC_CACHE = {}


def _get_nc(m, k, ns):
    key = (m, k, ns)
    if key not in _NC_CACHE:
        _NC_CACHE[key] = build_nc(m, k, ns)
    return _NC_CACHE[key]


def kernel(input, weight_scale, weight, group_size=GS, trace=False):
    m, k = input.shape
    n = weight.shape[0]
    ns = n // NCORES
    nc = _get_nc(m, k, ns)
    in_maps = host_prep(input, weight_scale, weight, ns)
    res = bass_utils.run_bass_kernel_spmd(
        nc, in_maps, core_ids=list(range(NCORES)), trace=trace
    )
    out = np.concatenate([r["out"] for r in res.results], axis=1)
    if trace:
        return out, res
    return out


if __name__ == "__main__":
    # small-config CoreSim check (full k so the schedule tables apply)
    from concourse.bass_interp import CoreSim

    rng = np.random.default_rng(0)
    m, k, ns = 256, 8192, 256
    x = rng.standard_normal((m, k), dtype=np.float32)
    w_tern = rng.integers(-1, 2, size=(ns, k)).astype(np.int32)
    codes = (w_tern + 1).reshape(ns, k // 4, 4)
    packed = (
        codes[..., 0] | (codes[..., 1] << 2) | (codes[..., 2] << 4)
        | (codes[..., 3] << 6)
    ).astype(np.int32)
    ws = rng.uniform(0.001, 0.02, size=(ns, k // GS, 1)).astype(np.float32)

    # numpy reference
    s = 127.0 / np.clip(np.abs(x).max(axis=-1, keepdims=True), 1e-5, None)
    q = np.clip(np.round(x * s), -128, 127)
    wf = w_tern.astype(np.float32) * np.repeat(ws.reshape(ns, -1), GS, axis=1)
    ref = ((q @ wf.T) / s).astype(ml_dtypes.bfloat16).astype(np.float32)

    nc = build_nc(m, k, ns)
    im = host_prep(x, ws, packed, ns)[0]
    sim = CoreSim(nc)
    for kk, v in im.items():
        sim.tensor(kk)[:] = v
    sim.simulate()
    got = np.asarray(sim.tensor("out")).astype(np.float32)
    err = np.abs(got - ref).max() / (np.abs(ref).max() + 1e-9)
    print("rel err (absmax):", err)
    rms = np.sqrt(((got - ref) ** 2).mean()) / (np.sqrt((ref**2).mean()) + 1e-9)
    print("rel err (rms):", rms)


# revision 18
# speedup vs baseline: 1.0571x; 1.0571x over previous
"""BitLinear (ternary 2-bit weights, group-128 scales, dynamic int8 activation
quant) for Trainium2, tensor-parallel over 8 NeuronCores (shard N).

Math (per core, N-shard NS):
  s[m]   = 127 / clip(max_k |x[m,k]|, 1e-5)
  q[m,k] = round(x[m,k] * s[m])                      (integers in [-127,127])
  w[n,k] in {-1,0,1} from 2-bit codes c=w+1, 4 codes/byte
  out[m,n] = (sum_k q[m,k] * w[n,k] * ws[n, k//128]) / s[m]   -> bf16

Pipelined schedule (v2): x chunks DMA first (tail chunks split for finer
arrival granularity); row abs-max runs chunk-pipelined on DVE interleaved
with the weight decode (tensor_scalar extract at 4x + tensor_tensor
scale-multiply at 2x; several multiplies offloaded to the otherwise-idle
Pool engine); quantization (ACT fma-magic RNE rounding) + PE
identity-transposes + main matmuls are pipelined per 2048-wide k-chunk so
the GEMM stream starts ~26us instead of ~50us.  The m-block mh=1 lags mh=0
and catches up inside the main stream.  Transposes run in is_transpose mode
into bf16 PSUM tiles, evacuated in [128,1024] batches into a single qp_all
buffer.  The "-1" correction (out -= gz @ ws.T, gz = per-group sums of q)
accumulates 8-column gmat matmuls in PE decode-stall slots, is transposed on
the PE, and lands as 4 bf16 matmuls into the persistent PSUM accumulators
mid-stream.  fp32 warm-up matmuls tied to x-chunk arrivals keep the PE HAM
activity monitor at 2.4GHz through the head.
"""

import sys

import numpy as np

try:
    import concourse.bass as bass
except ImportError:  # fresh grading dir: fall back to the repo checkout
    sys.path.insert(0, "/opt/trn_rl_repo")
    import concourse.bass as bass

import ml_dtypes

import concourse.mybir as mybir
import concourse.tile as tile
from concourse import bacc, bass_utils
from concourse.masks import make_identity

FP32 = mybir.dt.float32
BF16 = mybir.dt.bfloat16
U16 = mybir.dt.uint16
MAGIC = float(2 << 22)  # 2^23
COPY = mybir.ActivationFunctionType.Copy

M, N, K, GS = 256, 8192, 8192, 128
NCORES = 8


def build_nc(m=M, k=K, ns=N // NCORES):
    """One core's program: full m,k; n-shard of size ns."""
    kh = k // 8          # uint16 count along K
    kb = kh // 128       # kh-blocks of 128 partitions
    st_n = kb // 2       # supertiles = pairs of kh-blocks
    mt = m // 128        # m partition-tiles
    ck = k // 2048       # 2048-wide k-chunks (256 kh, 2 kh-blocks)
    nsl = min(512, ns)   # matmul rhs free-dim slice
    nh_n = ns // nsl
    g_n = k // GS        # scale groups along K
    nslots = st_n * 8    # linear (st, t) slot index space
    assert mt == 2 and ck == st_n == 4

    nc = bacc.Bacc()
    x_d = nc.declare_dram_parameter("x", [m, k], FP32, isOutput=False)
    w_d = nc.declare_dram_parameter("w16", [kh, ns], U16, isOutput=False)
    se_d = nc.declare_dram_parameter("sexp", [kh, ns], BF16, isOutput=False)
    # -ws.T in bf16 for the "-1" correction matmul (same bf16 rounding as
    # sexp so the w=0 contributions cancel exactly)
    sn_d = nc.declare_dram_parameter("sneg", [g_n, ns], BF16, isOutput=False)
    # G[kh, 8*t+gl] = (kh//16 == gl): group-sum indicator, bf16
    gm_d = nc.declare_dram_parameter("gmat", [128, 64], BF16, isOutput=False)
    out_d = nc.declare_dram_parameter("out", [m, ns], BF16, isOutput=True)

    x_r = x_d.rearrange("(T p) k -> T p k", p=128)          # [mt,128,k]
    w_r = w_d.rearrange("(B p) n -> p B n", p=128)          # [128,kb,ns]
    se_r = se_d.rearrange("(B p) n -> p B n", p=128)        # [128,kb,ns]
    out_r = out_d.rearrange("(T p) n -> T p n", p=128)      # [mt,128,ns]

    # Pool shares SBUF ports with DVE: concurrent Pool bulk ops slow DVE
    # ~6x (measured), so all decode work stays on DVE.

    with tile.TileContext(nc) as tc:
        with (
            tc.tile_pool(name="const", bufs=1) as constp,
            tc.tile_pool(name="stat", bufs=1) as statp,
            tc.tile_pool(name="xp", bufs=7) as xp,
            tc.tile_pool(name="t1p", bufs=2) as t1p,
            tc.tile_pool(name="qcp", bufs=2) as qcp,
            tc.tile_pool(name="wse", bufs=2) as wsep,
            tc.tile_pool(name="cpp", bufs=2) as cpp,
            tc.tile_pool(name="wsp", bufs=10) as wsp,
            tc.tile_pool(name="ob", bufs=2) as obp,
            tc.tile_pool(name="psm", bufs=1, space="PSUM") as psmp,
            tc.tile_pool(name="psT", bufs=2, space="PSUM") as psTp,
            tc.tile_pool(name="pss", bufs=1, space="PSUM") as pssp,
        ):
            # -------- DMA issues: x(0,*) first, st0 weights, then x(1,*) --
            xsb = {}

            def x_dma(mh, c, pieces=1):
                t_ = xp.tile([128, 2048], FP32, tag="x", name=f"x{mh}{c}")
                xsb[(mh, c)] = t_
                step = 2048 // pieces
                for p_ in range(pieces):
                    sl = slice(step * p_, step * (p_ + 1))
                    nc.sync.dma_start(
                        t_[:, sl], x_r[mh, :, 2048 * c + step * p_ :
                                       2048 * c + step * (p_ + 1)])

            wt_t, se_t = {}, {}

            def load_st(sti, eng=None):
                eng = eng or nc.sync
                wt = wsep.tile([128, 2 * ns], U16, tag="w16", name=f"wt{sti}")
                se = wsep.tile([128, 2 * ns], BF16, tag="sexp",
                               name=f"se{sti}")
                wt3 = wt.rearrange("p (B n) -> p B n", B=2)
                se3 = se.rearrange("p (B n) -> p B n", B=2)
                eng.dma_start(wt3[:], w_r[:, 2 * sti : 2 * sti + 2, :])
                eng.dma_start(se3[:], se_r[:, 2 * sti : 2 * sti + 2, :])
                wt_t[sti], se_t[sti] = wt, se

            x_dma(0, 0)
            x_dma(0, 1)
            x_dma(0, 2)
            x_dma(0, 3, pieces=4)
            load_st(0)
            x_dma(1, 0)
            x_dma(1, 1)
            x_dma(1, 2)
            x_dma(1, 3, pieces=2)
            sneg = constp.tile([g_n, ns], BF16, tag="sneg")
            nc.sync.dma_start(sneg[:], sn_d[:])
            gmat = constp.tile([128, 64], BF16, tag="gmat")
            nc.sync.dma_start(gmat[:], gm_d[:])
            load_st(1)
            load_st(2)
            load_st(3)

            # ---------------- constants ----------------------------------
            ident = constp.tile([128, 128], BF16, tag="ident")
            make_identity(nc, ident)

            # ------------- stats: row abs-max + recip, all on DVE ---------
            rpart = [statp.tile([128, 16], FP32, tag=f"rp{t}", name=f"rp{t}")
                     for t in range(mt)]
            rmax = [statp.tile([128, 1], FP32, tag=f"rm{t}", name=f"rm{t}")
                    for t in range(mt)]
            s_pp = [statp.tile([128, 1], FP32, tag=f"sp{t}", name=f"sp{t}")
                    for t in range(mt)]
            r1s = [statp.tile([128, 1], FP32, tag=f"rs{t}", name=f"rs{t}")
                   for t in range(mt)]
            rcols = [0, 0]

            def rm_chunk(mh, c, pieces=1):
                step = 2048 // pieces
                for h in range(pieces):
                    sl = slice(step * h, step * (h + 1))
                    nc.vector.tensor_reduce(
                        rpart[mh][:, rcols[mh] : rcols[mh] + 1],
                        xsb[(mh, c)][:, sl],
                        axis=mybir.AxisListType.X, op=mybir.AluOpType.max,
                        apply_absolute_value=True,
                    )
                    rcols[mh] += 1

            def stats_finish(mh):
                nc.vector.tensor_reduce(
                    rmax[mh][:], rpart[mh][:, : rcols[mh]],
                    axis=mybir.AxisListType.X, op=mybir.AluOpType.max,
                )
                nc.vector.tensor_scalar_max(rmax[mh][:], rmax[mh][:], 1e-5)
                nc.vector.reciprocal(s_pp[mh][:], rmax[mh][:])
                nc.vector.tensor_scalar_mul(s_pp[mh][:], s_pp[mh][:], 127.0)
                nc.vector.tensor_scalar_mul(r1s[mh][:], rmax[mh][:],
                                            1.0 / 127.0)

            # ---------------- ACT quant ----------------------------------
            qc = {}

            def quant(mh, c):
                q_ = qcp.tile([128, 2048], BF16, tag="qc", name=f"q{mh}{c}")
                qc[(mh, c)] = q_
                for h in range(2):
                    sl = slice(1024 * h, 1024 * (h + 1))
                    t1 = t1p.tile([128, 1024], FP32, tag="t1", name="t1")
                    nc.scalar.activation(t1[:], xsb[(mh, c)][:, sl], COPY,
                                         bias=MAGIC, scale=s_pp[mh][:])
                    nc.scalar.activation(q_[:, sl], t1[:], COPY, bias=-MAGIC)

            # ---------------- PE helpers ----------------------------------
            # qp_all[p, 2048*b+256*t+128*mh+mm] = q[128*mh+mm, 8*(128b+p)+t]
            qp_all = constp.tile([128, 2048 * kb], BF16, tag="qp")
            qp_r = qp_all.rearrange("p (b t m2 mm) -> p b t m2 mm",
                                    b=kb, t=8, m2=mt, mm=128)

            def warm_x(mh, c, n_mm=4):
                # scratch warms into the psgm bank (b2 resets it via start=T)
                for j in range(n_mm):
                    nc.tensor.matmul(psgm[:, :128], xsb[(mh, c)][:, :128],
                                     xsb[(mh, c)][:, :128],
                                     start=True, stop=True)

            def warm_q(mh, c, n_mm=2):
                for j in range(n_mm):
                    nc.tensor.matmul(psgm[:, :128], qc[(mh, c)][:, :128],
                                     ident[:], start=True, stop=True)

            def transpose_group(mh, c, bh2):
                """8 bit-plane transposes of one kh-block into bf16 PSUM,
                then one batched ACT evacuation into qp_all."""
                b = 2 * c + bh2
                psT = psTp.tile([128, 1024], BF16, tag="psT", name="psT")
                qv = qc[(mh, c)].rearrange("p (B kk t) -> p B t kk", B=2,
                                           kk=128, t=8)
                for t in range(8):
                    nc.tensor.transpose(psT[:, 128 * t : 128 * (t + 1)],
                                        qv[:, bh2, t, :], ident[:])
                psT3 = psT.rearrange("p (t mm) -> p t mm", t=8)
                nc.scalar.activation(qp_r[:, b, :, mh, :], psT3[:], COPY)

            def decode(sti, t):
                """c_t = (w16 >> 2t) & 3 on DVE (4x); ws_t = c_t * sexp."""
                cp = cpp.tile([128, 2 * ns], U16, tag="cp", name="cp")
                nc.vector.tensor_scalar(
                    cp[:], wt_t[sti][:], 2 * t, 3,
                    mybir.AluOpType.logical_shift_right,
                    mybir.AluOpType.bitwise_and,
                )
                ws = wsp.tile([128, 2 * ns], BF16, tag="ws", name="ws")
                nc.vector.tensor_tensor(ws[:], cp[:], se_t[sti][:],
                                        mybir.AluOpType.mult)
                ws_tiles[(sti, t)] = (ws, 0)

            # persistent PSUM accumulators
            psm = [
                [psmp.tile([128, nsl], FP32, tag=f"ps{mh}{nh}",
                           name=f"ps{mh}{nh}") for nh in range(nh_n)]
                for mh in range(mt)
            ]
            psgm = pssp.tile([128, 64 * mt], FP32, tag="psgm")
            gsqm = [constp.tile([128, 64], BF16, tag=f"gsqm{mh}",
                                name=f"gsqm{mh}") for mh in range(mt)]
            psg = pssp.tile([64, 128 * mt], BF16, tag="psg")
            gsq = constp.tile([64, 128 * mt], BF16, tag="gsq")

            ws_tiles = {}

            def main_mms(sti, t, mh, stop=False):
                ws, off = ws_tiles[(sti, t)]
                for bh in range(2):
                    b = 2 * sti + bh
                    first = sti == 0 and t == 0 and bh == 0
                    last = stop and bh == 1
                    for nh in range(nh_n):
                        nc.tensor.matmul(
                            psm[mh][nh][:],
                            qp_r[:, b, t, mh, :],
                            ws[:, off + ns * bh + nsl * nh :][:, :nsl],
                            start=first, stop=last,
                        )

            def b2_unit(mh, b):
                for t in range(8):
                    nc.tensor.matmul(
                        psgm[:, 64 * mh + 8 * b : 64 * mh + 8 * b + 8],
                        qp_r[:, b, t, mh, :], gmat[:, 8 * t : 8 * t + 8],
                        start=(t == 0), stop=(t == 7),
                    )

            def corr_chain():
                # psgm -> bf16 -> PE transpose -> gsq[g, m]; then 4 bf16
                # matmuls accumulate -gz @ ws.T into psm (start=F, stop=F)
                for mh in range(mt):
                    nc.scalar.activation(gsqm[mh][:],
                                         psgm[:, 64 * mh : 64 * mh + 64],
                                         COPY)
                    nc.tensor.transpose(psg[:, 128 * mh : 128 * (mh + 1)],
                                        gsqm[mh][:], ident[:])
                nc.scalar.activation(gsq[:], psg[:], COPY)
                for mh in range(mt):
                    for nh in range(nh_n):
                        nc.tensor.matmul(
                            psm[mh][nh][:],
                            gsq[:, 128 * mh : 128 * (mh + 1)],
                            sneg[:, nsl * nh :][:, :nsl],
                            start=False, stop=False,
                        )

            def out_evac(mh):
                for nh in range(nh_n):
                    ob = obp.tile([128, nsl], BF16, tag="ob", name="ob")
                    nc.scalar.activation(ob[:], psm[mh][nh][:], COPY,
                                         scale=r1s[mh][:])
                    nc.sync.dma_start(
                        out_r[mh, :, nsl * nh : nsl * (nh + 1)], ob[:])

            # ---------------- head emission -------------------------------
            # DVE: mh0 rowmax chunk-pipelined, stats, first decode
            rm_chunk(0, 0)
            rm_chunk(0, 1)
            rm_chunk(0, 2)
            rm_chunk(0, 3, pieces=4)
            stats_finish(0)
            # ACT: quantize chunk 0; PE: warms + first transposes
            quant(0, 0)
            for c in range(ck):
                warm_x(0, c, 3)
            warm_q(0, 0, 2)
            transpose_group(0, 0, 0)
            transpose_group(0, 0, 1)
            quant(0, 1)
            decode(0, 0)

            # ---------------- schedule tables (lin = 8*st + t) ------------
            pe_prep = {3: [(0, 1)], 6: [(0, 2)], 9: [(0, 3)],
                       11: [(1, 0)], 14: [(1, 1)], 17: [(1, 2)],
                       20: [(1, 3)]}
            act_prep = {1: [(0, 2)], 4: [(0, 3)], 7: [(1, 0)],
                        10: [(1, 1)], 12: [(1, 2)], 14: [(1, 3)]}
            dve_prep = {0: [(1, 0, 1)], 1: [(1, 1, 1)], 2: [(1, 2, 1)],
                        3: [(1, 3, 2)]}
            b2_sched = {4: [(0, 0)], 5: [(0, 1)], 7: [(0, 2)],
                        8: [(0, 3)], 10: [(0, 4)], 11: [(0, 5)],
                        13: [(0, 6), (0, 7)],
                        15: [(1, 0), (1, 1)], 18: [(1, 2), (1, 3)],
                        21: [(1, 4), (1, 5)], 23: [(1, 6), (1, 7)]}
            corr_lin = 24

            mh1_units = [(st, t) for st in range(st_n) for t in range(8)]
            mh1_done = 0
            tgroups1 = [0]

            def mh1_target(lin):
                if lin < 11:
                    return 0
                return min(nslots, -(-(lin - 10) * nslots // 21), lin + 1,
                           8 * tgroups1[0])

            def emit_mh1(upto):
                nonlocal mh1_done
                while mh1_done < upto:
                    st_, t_ = mh1_units[mh1_done]
                    main_mms(st_, t_, 1,
                             stop=(st_ == st_n - 1 and t_ == 7))
                    mh1_done += 1

            # ---------------- main loop -----------------------------------
            for lin in range(nslots):
                sti, t = lin // 8, lin % 8
                for (mh_, c_, pieces) in dve_prep.get(lin, ()):
                    rm_chunk(mh_, c_, pieces)
                    if (mh_, c_) == (1, 3):
                        stats_finish(1)
                for (mh_, c_) in act_prep.get(lin, ()):
                    quant(mh_, c_)
                for (mh_, c_) in pe_prep.get(lin, ()):
                    transpose_group(mh_, c_, 0)
                    transpose_group(mh_, c_, 1)
                    if mh_ == 1:
                        tgroups1[0] += 1
                if (sti, t) not in ws_tiles:
                    decode(sti, t)
                main_mms(sti, t, 0, stop=(sti == st_n - 1 and t == 7))
                for (mh_, b_) in b2_sched.get(lin, ()):
                    b2_unit(mh_, b_)
                if lin == corr_lin:
                    corr_chain()
                emit_mh1(mh1_target(lin))
            out_evac(0)   # psm0 stopped at (st3,t7,mh0); overlaps trailing
            emit_mh1(nslots)
            out_evac(1)

    nc.compile()
    return nc


def host_prep(input, weight_scale, weight, ns):
    """Shard + relayout inputs for each core. Pure relayout of static weight
    data (transpose, uint8->uint16 view, group-scale expansion) plus fp32
    activation passthrough."""
    n = weight.shape[0]
    x = np.ascontiguousarray(input, dtype=np.float32)
    w_bytes = weight.astype(np.uint8)              # [N, K/4] packed bytes
    w16 = w_bytes.view(np.uint16)                  # [N, K/8] 8 codes each
    ws2 = np.asarray(weight_scale, dtype=np.float32).reshape(n, -1)  # [N,K/GS]
    ws2_b = ws2.astype(ml_dtypes.bfloat16)
    # gmat[kh, 8*t+gl] = (kh//16 == gl)  group-sum indicator
    gmat = np.zeros((128, 64), dtype=np.float32)
    for t in range(8):
        for khp in range(128):
            gmat[khp, 8 * t + khp // 16] = 1.0
    gmat = gmat.astype(ml_dtypes.bfloat16)
    in_maps = []
    for c in range(n // ns):
        sl = slice(c * ns, (c + 1) * ns)
        w16_c = np.ascontiguousarray(w16[sl].T)    # [KH, ns]
        se_c = np.ascontiguousarray(ws2_b[sl].T.repeat(16, axis=0))  # [KH,ns]
        sn_c = -np.ascontiguousarray(ws2_b[sl].T)  # [K/GS, ns] bf16
        in_maps.append(
            {"x": x, "w16": w16_c, "sexp": se_c, "sneg": sn_c, "gmat": gmat}
        )
    return in_maps


_NC_CACHE = {}


def _get_nc(m, k, ns):
    key = (m, k, ns)
    if key not in _NC_CACHE:
        _NC_CACHE[key] = build_nc(m, k, ns)
    return _NC_CACHE[key]


def kernel(input, weight_scale, weight, group_size=GS, trace=False):
    m, k = input.shape
    n = weight.shape[0]
    ns = n // NCORES
    nc = _get_nc(m, k, ns)
    in_maps = host_prep(input, weight_scale, weight, ns)
    res = bass_utils.run_bass_kernel_spmd(
        nc, in_maps, core_ids=list(range(NCORES)), trace=trace
    )
    out = np.concatenate([r["out"] for r in res.results], axis=1)
    if trace:
        return out, res
    return out


if __name__ == "__main__":
    # small-config CoreSim check (full k so the schedule tables apply)
    from concourse.bass_interp import CoreSim

    rng = np.random.default_rng(0)
    m, k, ns = 256, 8192, 256
    x = rng.standard_normal((m, k), dtype=np.float32)
    w_tern = rng.integers(-1, 2, size=(ns, k)).astype(np.int32)
    codes = (w_tern + 1).reshape(ns, k // 4, 4)
    packed = (
        codes[..., 0] | (codes[..., 1] << 2) | (codes[..., 2] << 4)
        | (codes[..., 3] << 6)
    ).astype(np.int32)
    ws = rng.uniform(0.001, 0.02, size=(ns, k // GS, 1)).astype(np.float32)

    # numpy reference
    s = 127.0 / np.clip(np.abs(x).max(axis=-1, keepdims=True), 1e-5, None)
    q = np.clip(np.round(x * s), -128, 127)
    wf = w_tern.astype(np.float32) * np.repeat(ws.reshape(ns, -1), GS, axis=1)
    ref = ((q @ wf.T) / s).astype(ml_dtypes.bfloat16).astype(np.float32)

    nc = build_nc(m, k, ns)
    im = host_prep(x, ws, packed, ns)[0]
    sim = CoreSim(nc)
    for kk, v in im.items():
        sim.tensor(kk)[:] = v
    sim.simulate()
    got = np.asarray(sim.tensor("out")).astype(np.float32)
    err = np.abs(got - ref).max() / (np.abs(ref).max() + 1e-9)
    print("rel err (absmax):", err)
    rms = np.sqrt(((got - ref) ** 2).mean()) / (np.sqrt((ref**2).mean()) + 1e-9)
    print("rel err (rms):", rms)
